# revision 14
# baseline (speedup 1.0000x reference)
"""CLVP attention kernel for 8 Trainium2 NeuronCores.

Problem: B=2, T=2048, E=768, H=12 heads of HD=64; rotary on first 32 dims
of q, k AND v; softmax attention; output projection.

Sharding: the 24 (batch, head) pairs are split 3-heads-x-1-batch per core
(core c: batch c//4, heads 3*(c%4)..3*(c%4)+2).  Wq/Wk/Wv are split
column-wise (by head), Wo row-wise, so each core produces a partial
(T, E) output for its batch; the host sums the 4 partials per batch
(row-parallel tensor parallelism) and adds bo.

Per-core device program:
  - inputs (host-prepped layouts): xT (768,2048) = hidden[b].T,
    wqk (768,384) = [Wq_rows.T * scale | Wk_rows.T], wv (768,256) =
    Wv_rows.T zero-padded, wo (192,768) = Wo[:,cols].T, fr (128,512) =
    rotary packed as (t%128, t//128, 32).
  - qkv projection in natural layout (t on partitions) via matmuls
  - RoPE applied with strided-AP DVE ops (heads batched in the free dim)
  - q,k transposed to (d, t) layout on the PE (via identity matmul)
  - per head: scoresT = k_h.T-major matmul -> PSUM, exp on ACT
    (PSUM->SBUF), out.T accumulated as v_ones.T @ E where v carries an
    extra ones column -> row 64 of the accumulator is the softmax
    denominator for free; normalization deferred to after the matmul.
  - projection y = out_heads @ woT accumulated over heads, DMA'd from
    PSUM straight to DRAM.
"""

import numpy as np

try:
    import concourse.bass as bass
except ImportError:
    import sys
    for p in ("/opt/trn_rl_repo", "/root/.axon_site/_ro/trn_rl_repo"):
        if p not in sys.path:
            sys.path.insert(0, p)
    import concourse.bass as bass

import ml_dtypes
import concourse.tile as tile
from concourse import bacc, mybir, bass_utils
from concourse.masks import make_identity

F32 = mybir.dt.float32
F32R = mybir.dt.float32r
F16 = mybir.dt.float16
BF16 = mybir.dt.bfloat16
AF = mybir.ActivationFunctionType
OP = mybir.AluOpType

B, T, E, H, HD, ROT = 2, 2048, 768, 12, 64, 32
SCALE = HD ** -0.5
HPC = 3                      # heads per core
NC_CORES = 8
TCH = T // 128               # 16 t-chunks
ECH = E // 128               # 6 embedding chunks
QK = 2 * HPC * HD            # 384  (q rows + k rows per core)
VW = HPC * (HD + 1)          # 195  (v + ones column per head) per chunk

# --- dtype knobs (probing which PE/engine paths are fast on this HW) ---
XDT = F32R     # xt / wqk / wv: qkv-projection matmul operands
NATDT = F32R   # qk_nat + ident + transpose PSUM tile
QKTDT = F32R   # qT/kT tiles: scores matmul operands
EMDT = F32R    # e_m (exp output; out-accum rhs) -- keep == VTDT
VTDT = F32R    # vt (out-accum lhsT)
ODT = F32R     # oT tiles + wo: projection matmul operands
V_ROPE_ENG = "gpsimd"   # engine for the v rope
ATTN_REPS = 8  # probe: replicate phase E (idempotent) to calibrate timing

_NP = {F32R: np.float32, F32: np.float32, BF16: ml_dtypes.bfloat16,
       F16: np.float16}


def _v(t, offset, dims):
    """AP view of tile t at element offset with free dims [[step, count], ...]."""
    return bass.AP(tensor=t.tensor, offset=t.offset + offset,
                   ap=[t.ap[0]] + [list(d) for d in dims])


def _load(nc, out, dram_ap, dt):
    """DMA dram->sbuf; bitcast when the dram tensor is declared f32 but the
    tile is f32r (same bits)."""
    if dt == F32R:
        nc.sync.dma_start(out=out, in_=dram_ap.bitcast(F32R))
    else:
        nc.sync.dma_start(out=out, in_=dram_ap)


def build_nc():
    nc = bacc.Bacc("TRN2", target_bir_lowering=False, debug=False)

    def dram_dt(dt):
        return F32 if dt == F32R else dt

    xT_d = nc.dram_tensor("xT", [E, T], dram_dt(XDT), kind="ExternalInput").ap()
    wqk_d = nc.dram_tensor("wqk", [E, QK], dram_dt(XDT), kind="ExternalInput").ap()
    wv_d = nc.dram_tensor("wv", [E, 256], dram_dt(XDT), kind="ExternalInput").ap()
    wo_d = nc.dram_tensor("wo", [HPC * HD, E], dram_dt(ODT), kind="ExternalInput").ap()
    fr_d = nc.dram_tensor("fr", [128, TCH * ROT], F32, kind="ExternalInput").ap()
    y_d = nc.dram_tensor("y", [T, E], F32, kind="ExternalOutput").ap()

    with tile.TileContext(nc) as tc:
        with tc.tile_pool(name="persist", bufs=1) as P, \
             tc.tile_pool(name="dram", bufs=1, space="DRAM") as DR:
            # ---- persistent SBUF tiles ----
            wqk_t = P.tile([128, ECH, QK], XDT)      # wqk as 6 chunks
            wv_t = P.tile([128, ECH, 256], XDT)
            wo_a = P.tile([128, E], ODT)             # heads 0,1 of woT
            wo_b = P.tile([64, E], ODT)              # head 2
            fr_t = P.tile([128, TCH * ROT], F32)
            cos_t = P.tile([128, TCH * ROT], F32)
            sin_t = P.tile([128, TCH * ROT], F32)
            ident = P.tile([128, 128], NATDT)
            ident_f = P.tile([128, 128], F32)
            halfpi = P.tile([128, 1], F32)
            qk_nat = P.tile([128, TCH * QK], NATDT)  # q|k natural, 6144
            vt = P.tile([128, TCH * VW], VTDT)       # v+ones, 3120
            qT_a = P.tile([128, T], QKTDT)           # q heads 0,1 (d-major)
            qT_b = P.tile([64, T], QKTDT)
            kT_a = P.tile([128, T], QKTDT)
            kT_b = P.tile([64, T], QKTDT)
            oT_a = P.tile([128, T], ODT)             # normalized out.T h0,h1
            oT_b = P.tile([64, T], ODT)
            dr_bounce = DR.tile([HPC, T], F32)

            # ---- loads ----
            for j in range(ECH):
                _load(nc, wqk_t[:, j, :], wqk_d[j * 128:(j + 1) * 128, :], XDT)
                _load(nc, wv_t[:, j, :], wv_d[j * 128:(j + 1) * 128, :], XDT)
            make_identity(nc, ident_f)
            nc.vector.tensor_copy(ident, ident_f)
            nc.gpsimd.memset(halfpi, float(np.pi / 2))
            # ones columns of vt: write the 1.0 bit pattern for its dtype
            if VTDT in (F32R, F32):
                nc.gpsimd.memset(vt.bitcast(mybir.dt.uint32), 0x3F800000)
            else:
                nc.gpsimd.memset(vt.bitcast(mybir.dt.uint16),
                                 0x3F80 if VTDT == BF16 else 0x3C00)

            # ---- phases C+D: qkv projection, RoPE, transposes, pipelined
            # per t-quarter so PE/DVE/Pool overlap ----
            with tc.tile_pool(name="xt", bufs=1) as XP, \
                 tc.tile_pool(name="rope_scr", bufs=2) as RS, \
                 tc.tile_pool(name="psC", bufs=3, space="PSUM") as PSC, \
                 tc.tile_pool(name="psD", bufs=2, space="PSUM") as PSD:
                xt = XP.tile([128, ECH, T], XDT)

                def rope_q(eng, base_t, blk_w, nblk, q, scr):
                    h = 16
                    A = scr.tile([128, 4 * nblk * h], F32, tag=f"ropeA{blk_w}")
                    Bv = scr.tile([128, 4 * nblk * h], F32, tag=f"ropeB{blk_w}")
                    off = q * 4 * blk_w * nblk
                    Q1 = _v(base_t, off, [[blk_w * nblk, 4], [blk_w, nblk], [1, h]])
                    Q2 = _v(base_t, off + h, [[blk_w * nblk, 4], [blk_w, nblk], [1, h]])
                    Av = _v(A, 0, [[nblk * h, 4], [h, nblk], [1, h]])
                    Bvv = _v(Bv, 0, [[nblk * h, 4], [h, nblk], [1, h]])
                    fo = q * 4 * ROT
                    C1 = _v(cos_t, fo, [[ROT, 4], [0, nblk], [1, h]])
                    C2 = _v(cos_t, fo + h, [[ROT, 4], [0, nblk], [1, h]])
                    S1 = _v(sin_t, fo, [[ROT, 4], [0, nblk], [1, h]])
                    S2 = _v(sin_t, fo + h, [[ROT, 4], [0, nblk], [1, h]])
                    eng.tensor_tensor(Av, Q2, S1, OP.mult)
                    eng.tensor_tensor(Bvv, Q1, S2, OP.mult)
                    eng.tensor_tensor(Q1, Q1, C1, OP.mult)
                    eng.tensor_tensor(Q1, Q1, Av, OP.subtract)
                    eng.tensor_tensor(Q2, Q2, C2, OP.mult)
                    eng.tensor_tensor(Q2, Q2, Bvv, OP.add)

                for q in range(4):
                    if q == 0:
                        # first quarter in two half-waves so chunk 0 starts
                        # after ~0.75MB instead of 1.5MB; fr/wo loads deferred
                        # out of the critical initial burst
                        for w in range(2):
                            wsl = slice(w * 256, (w + 1) * 256)
                            for j in range(ECH):
                                _load(nc, xt[:, j, wsl],
                                      xT_d[j * 128:(j + 1) * 128, wsl], XDT)
                            if w == 0:
                                nc.sync.dma_start(out=fr_t, in_=fr_d)
                                # cos/sin tables (emitted after the fr load:
                                # Tile deps follow emission order). fr in
                                # [0,1) so fr + pi/2 stays in [-pi, pi]
                                nc.scalar.activation(cos_t, fr_t, AF.Sin,
                                                     bias=halfpi[:, :], scale=1.0)
                                nc.scalar.activation(sin_t, fr_t, AF.Sin,
                                                     scale=1.0)
                    else:
                        tsl = slice(q * 512, (q + 1) * 512)
                        for j in range(ECH):
                            _load(nc, xt[:, j, tsl],
                                  xT_d[j * 128:(j + 1) * 128, tsl], XDT)
                    if q == 1:
                        _load(nc, wo_a, wo_d[0:128, :], ODT)
                        _load(nc, wo_b, wo_d[128:192, :], ODT)
                    for i in range(4 * q, 4 * q + 4):
                        ps_qk = PSC.tile([128, QK], F32, tag="qk")
                        ps_v = PSC.tile([128, 256], F32, tag="v")
                        for j in range(ECH):
                            lhs = xt[:, j, i * 128:(i + 1) * 128]
                            nc.tensor.matmul(ps_qk, lhs, wqk_t[:, j, :],
                                             start=(j == 0), stop=(j == ECH - 1))
                            nc.tensor.matmul(ps_v, lhs, wv_t[:, j, :],
                                             start=(j == 0), stop=(j == ECH - 1))
                        nc.any.tensor_copy(qk_nat[:, i * QK:(i + 1) * QK], ps_qk)
                        # v columns into 65-wide head blocks (ones col untouched)
                        dst = _v(vt, i * VW, [[HD + 1, HPC], [1, HD]])
                        src = _v(ps_v, 0, [[HD, HPC], [1, HD]])
                        nc.any.tensor_copy(dst, src)

                    rope_q(nc.vector, qk_nat, HD, 2 * HPC, q, RS)
                    rope_q(getattr(nc, V_ROPE_ENG), vt, HD + 1, HPC, q, RS)

                    for i in range(4 * q, 4 * q + 4):
                        col = i * QK
                        ts_sl = slice(i * 128, (i + 1) * 128)
                        pt = PSD.tile([128, 512], NATDT, tag="tr")
                        nc.tensor.transpose(pt[:, 0:128], qk_nat[:, col:col + 128], ident)
                        nc.tensor.transpose(pt[0:64, 128:256], qk_nat[:, col + 128:col + 192], ident)
                        nc.tensor.transpose(pt[:, 256:384], qk_nat[:, col + 192:col + 320], ident)
                        nc.tensor.transpose(pt[0:64, 384:512], qk_nat[:, col + 320:col + 384], ident)
                        nc.any.tensor_copy(qT_a[:, ts_sl], pt[:, 0:128])
                        nc.any.tensor_copy(qT_b[:, ts_sl], pt[0:64, 128:256])
                        nc.any.tensor_copy(kT_a[:, ts_sl], pt[:, 256:384])
                        nc.any.tensor_copy(kT_b[:, ts_sl], pt[0:64, 384:512])

            # ---- phase E: attention per head ----
            with tc.tile_pool(name="psS", bufs=2, space="PSUM") as PSS, \
                 tc.tile_pool(name="psO", bufs=2, space="PSUM") as PSO, \
                 tc.tile_pool(name="epool", bufs=4) as EP, \
                 tc.tile_pool(name="rcp", bufs=4) as RCP:
              for _rep in range(ATTN_REPS):
                for h in range(HPC):
                    if h == 0:
                        qT_h, kT_h = qT_a[0:64, :], kT_a[0:64, :]
                        oT_dst = oT_a[0:64, :]
                    elif h == 1:
                        qT_h, kT_h = qT_a[64:128, :], kT_a[64:128, :]
                        oT_dst = oT_a[64:128, :]
                    else:
                        qT_h, kT_h = qT_b[0:64, :], kT_b[0:64, :]
                        oT_dst = oT_b[0:64, :]

                    for half in range(2):
                        hoff = half * (T // 2)
                        ps_o = PSO.tile([HD + 1, T // 2], F32, tag="po")
                        for m in range(TCH):
                            ps_s = PSS.tile([128, T // 2], F32, tag="ps")
                            for n in range(2):
                                nsl = slice(n * 512, (n + 1) * 512)
                                nc.tensor.matmul(
                                    ps_s[:, nsl],
                                    kT_h[:, m * 128:(m + 1) * 128],
                                    qT_h[:, hoff + n * 512:hoff + (n + 1) * 512],
                                    start=True, stop=True)
                            e_m = EP.tile([128, T // 2], EMDT, tag="e")
                            nc.scalar.activation(e_m, ps_s, AF.Exp)
                            vh = _v(vt, m * VW + h * (HD + 1), [[1, HD + 1]])
                            for n in range(2):
                                nc.tensor.matmul(
                                    ps_o[:, n * 512:(n + 1) * 512],
                                    vh, e_m[:, n * 512:(n + 1) * 512],
                                    start=(m == 0), stop=(m == TCH - 1))

                        # per-half deferred normalization: copy raw out.T and
                        # reciprocal of the ones-row denominator, freeing ps_o
                        # early; the partition-broadcast goes through a DRAM
                        # bounce buffer.
                        hsl = slice(hoff, hoff + T // 2)
                        recip_h = RCP.tile([1, T // 2], F32, tag="rc")
                        nc.vector.reciprocal(recip_h, ps_o[HD:HD + 1, :])
                        nc.vector.tensor_copy(oT_dst[:, hsl], ps_o[0:HD, :])
                        nc.sync.dma_start(out=dr_bounce[h:h + 1, hsl], in_=recip_h)
                        # broadcast into the partition range matching oT_dst's
                        # base partition (DVE needs equal SB base partitions)
                        pbase = 64 * (h % 2)
                        rb_h = RCP.tile([128, T // 2], F32, tag="rb")
                        rb_src = bass.AP(tensor=dr_bounce.tensor,
                                         offset=dr_bounce.offset + h * T + hoff,
                                         ap=[[0, 64], [1, T // 2]])
                        nc.sync.dma_start(out=rb_h[pbase:pbase + 64, :], in_=rb_src)
                        nc.vector.tensor_tensor(oT_dst[:, hsl], oT_dst[:, hsl],
                                                rb_h[pbase:pbase + 64, :], OP.mult)

            # ---- phase F: output projection ----
            with tc.tile_pool(name="psY", bufs=4, space="PSUM") as PSY, \
                 tc.tile_pool(name="ysb", bufs=4) as YSB:
                for m in range(TCH):
                    msl = slice(m * 128, (m + 1) * 128)
                    ps_y = PSY.tile([128, E], F32, tag="y")
                    for nsl in (slice(0, 512), slice(512, E)):
                        nc.tensor.matmul(ps_y[:, nsl], oT_a[:, msl],
                                         wo_a[:, nsl], start=True, stop=False)
                        nc.tensor.matmul(ps_y[:, nsl], oT_b[:, msl],
                                         wo_b[:, nsl], start=False, stop=True)
                    y_s = YSB.tile([128, E], F32, tag="ys")
                    nc.vector.tensor_copy(y_s[:, 0:384], ps_y[:, 0:384])
                    nc.scalar.copy(y_s[:, 384:E], ps_y[:, 384:E])
                    nc.sync.dma_start(out=y_d[msl, :], in_=y_s)

    nc.compile()
    return nc


_NC_CACHE = None


def _get_nc():
    global _NC_CACHE
    if _NC_CACHE is None:
        _NC_CACHE = build_nc()
    return _NC_CACHE


def make_in_maps(hidden_states, rotary_pos_emb, Wq, Wk, Wv, Wo):
    fr = np.ascontiguousarray(
        rotary_pos_emb.reshape(TCH, 128, ROT).transpose(1, 0, 2).reshape(128, TCH * ROT),
        dtype=np.float32)
    xdt, odt = _NP[XDT], _NP[ODT]
    xT_by_b = [np.ascontiguousarray(hidden_states[b].T).astype(xdt)
               for b in range(B)]
    in_maps = []
    for c in range(NC_CORES):
        b = c // (NC_CORES // B)
        g = c % (NC_CORES // B)
        rows = slice(HPC * HD * g, HPC * HD * (g + 1))
        wqk = np.concatenate([Wq[rows].T * SCALE, Wk[rows].T], axis=1)
        wv = np.zeros((E, 256), dtype=xdt)
        wv[:, :HPC * HD] = Wv[rows].T.astype(xdt)
        wo = np.ascontiguousarray(Wo[:, rows].T).astype(odt)
        in_maps.append({
            "xT": xT_by_b[b],
            "wqk": np.ascontiguousarray(wqk.astype(xdt)),
            "wv": wv,
            "wo": wo,
            "fr": fr,
        })
    return in_maps


def kernel(hidden_states, rotary_pos_emb, Wq, Wk, Wv, Wo, bo):
    hidden_states = np.asarray(hidden_states, dtype=np.float32)
    rotary_pos_emb = np.asarray(rotary_pos_emb, dtype=np.float32)
    Wq = np.asarray(Wq, dtype=np.float32)
    Wk = np.asarray(Wk, dtype=np.float32)
    Wv = np.asarray(Wv, dtype=np.float32)
    Wo = np.asarray(Wo, dtype=np.float32)
    bo = np.asarray(bo, dtype=np.float32)

    from concourse import bass_utils as _bu
    nc = _get_nc()
    in_maps = make_in_maps(hidden_states, rotary_pos_emb.reshape(T, ROT),
                           Wq, Wk, Wv, Wo)
    res = _bu.run_bass_kernel_spmd(
        nc, in_maps, core_ids=list(range(NC_CORES)), trace=False)

    out = np.zeros((B, T, E), dtype=np.float32)
    for c in range(NC_CORES):
        out[c // (NC_CORES // B)] += res.results[c]["y"]
    out += bo
    return out


# revision 36
# speedup vs baseline: 52.2415x; 52.2415x over previous
"""CLVP attention kernel for 8 Trainium2 NeuronCores.

Problem: B=2, T=2048, E=768, H=12 heads of HD=64; rotary on first 32 dims
of q, k AND v; softmax attention; output projection.

Sharding: the 24 (batch, head) pairs are split 3-heads-x-1-batch per core
(core c: batch c//4, heads 3*(c%4)..3*(c%4)+2).  Wq/Wk/Wv are split
column-wise (by head), Wo row-wise, so each core produces a partial
(T, E) output for its batch; the host sums the 4 partials per batch
(row-parallel tensor parallelism) and adds bo.

Per-core device program:
  - inputs (host-prepped layouts): xT (768,2048) = hidden[b].T,
    wqk (768,384) = [Wq_rows.T * scale | Wk_rows.T], wv (768,256) =
    Wv_rows.T zero-padded, wo (192,768) = Wo[:,cols].T, fr (128,512) =
    rotary packed as (t%128, t//128, 32).
  - qkv projection in natural layout (t on partitions) via f32r matmuls
  - RoPE applied with strided-AP DVE ops (heads batched in the free dim)
  - q,k transposed to (d, t) layout on the PE (via identity matmul)
  - per head: scoresT = k_h.T-major matmul -> PSUM, exp on ACT
    (PSUM->SBUF), out.T accumulated as v_ones.T @ E where v carries an
    extra ones column -> row 64 of the accumulator is the softmax
    denominator for free; normalization deferred to after the matmul.
  - projection y = out_heads @ woT accumulated over heads, DMA'd from
    PSUM straight to DRAM.

Dtype note: f32r is the fastest matmul dtype on this stack (~1 cycle/row
for moving dims >= 256); bf16/fp16 measured ~1.2/2.2 cycles/row SLOWER
end-to-end, so everything stays f32r.
"""

import numpy as np

try:
    import concourse.bass as bass
except ImportError:
    import sys
    for p in ("/opt/trn_rl_repo", "/root/.axon_site/_ro/trn_rl_repo"):
        if p not in sys.path:
            sys.path.insert(0, p)
    import concourse.bass as bass

import concourse.tile as tile
from concourse import bacc, mybir, bass_utils
from concourse.masks import make_identity

F32 = mybir.dt.float32
F32R = mybir.dt.float32r
AF = mybir.ActivationFunctionType
OP = mybir.AluOpType

B, T, E, H, HD, ROT = 2, 2048, 768, 12, 64, 32
SCALE = HD ** -0.5
HPC = 3                      # heads per core
NC_CORES = 8
TCH = T // 128               # 16 t-chunks
ECH = E // 128               # 6 embedding chunks
QK = 2 * HPC * HD            # 384  (q rows + k rows per core)
VW = HPC * (HD + 1)          # 195  (v + ones column per head) per chunk

USE_F32R = True
DT = F32R if USE_F32R else F32


def _v(t, offset, dims):
    """AP view of tile t at element offset with free dims [[step, count], ...]."""
    return bass.AP(tensor=t.tensor, offset=t.offset + offset,
                   ap=[t.ap[0]] + [list(d) for d in dims])


def build_nc(whole_reps=1):
    """whole_reps>1 repeats the entire (idempotent) program body; used by
    test.py to measure steady-state device exec time via the slope between
    two rep counts, cancelling per-call host dispatch overhead."""
    nc = bacc.Bacc("TRN2", target_bir_lowering=False, debug=False)

    xT_d = nc.dram_tensor("xT", [E, T], F32, kind="ExternalInput").ap()
    wqk_d = nc.dram_tensor("wqk", [E, QK], F32, kind="ExternalInput").ap()
    wv_d = nc.dram_tensor("wv", [E, 256], F32, kind="ExternalInput").ap()
    wo_d = nc.dram_tensor("wo", [HPC * HD, E], F32, kind="ExternalInput").ap()
    fr_d = nc.dram_tensor("fr", [128, TCH * ROT], F32, kind="ExternalInput").ap()
    y_d = nc.dram_tensor("y", [T, E], F32, kind="ExternalOutput").ap()

    with tile.TileContext(nc) as tc:
      for _whole_rep in range(whole_reps):
        with tc.tile_pool(name="persist", bufs=1) as P, \
             tc.tile_pool(name="dram", bufs=1, space="DRAM") as DR:
            # ---- persistent SBUF tiles ----
            wqk_t = P.tile([128, ECH, QK], DT)      # wqk as 6 chunks
            wv_t = P.tile([128, ECH, 256], DT)
            wo_a = P.tile([128, E], DT)             # heads 0,1 of woT
            wo_b = P.tile([64, E], DT)              # head 2
            fr_t = P.tile([128, TCH * ROT], F32)
            cos_t = P.tile([128, TCH * ROT], F32)
            sin_t = P.tile([128, TCH * ROT], F32)
            ident = P.tile([128, 128], DT)
            ident_f = P.tile([128, 128], F32)
            halfpi = P.tile([128, 1], F32)
            qk_nat = P.tile([128, TCH * QK], DT)    # q|k natural, 6144
            vt = P.tile([128, TCH * VW], DT)        # v+ones, 3120
            qT_a = P.tile([128, T], DT)             # q heads 0,1 (d-major)
            qT_b = P.tile([64, T], DT)
            kT_a = P.tile([128, T], DT)
            kT_b = P.tile([64, T], DT)
            oT_a = P.tile([128, T], DT)             # normalized out.T h0,h1
            oT_b = P.tile([64, T], DT)
            dr_bounce = DR.tile([HPC, T], F32)

            # ---- loads ----
            for j in range(ECH):
                nc.sync.dma_start(out=wqk_t[:, j, :], in_=wqk_d[j * 128:(j + 1) * 128, :].bitcast(DT))
                nc.sync.dma_start(out=wv_t[:, j, :], in_=wv_d[j * 128:(j + 1) * 128, :].bitcast(DT))
            make_identity(nc, ident_f)
            nc.vector.tensor_copy(ident, ident_f)
            nc.gpsimd.memset(halfpi, float(np.pi / 2))
            # memset on f32r is not ISA-legal; write 1.0f bit pattern
            nc.gpsimd.memset(vt.bitcast(mybir.dt.uint32), 0x3F800000)

            # ---- phases C+D: qkv projection, RoPE, transposes, pipelined
            # per t-quarter so PE/DVE/Pool overlap ----
            with tc.tile_pool(name="xt", bufs=1) as XP, \
                 tc.tile_pool(name="rope_scr", bufs=2) as RS, \
                 tc.tile_pool(name="psC", bufs=3, space="PSUM") as PSC, \
                 tc.tile_pool(name="psD", bufs=2, space="PSUM") as PSD:
                xt = XP.tile([128, ECH, T], DT)

                def rope_q(eng, base_t, blk_w, nblk, q, scr):
                    h = 16
                    A = scr.tile([128, 4 * nblk * h], F32, tag=f"ropeA{blk_w}")
                    Bv = scr.tile([128, 4 * nblk * h], F32, tag=f"ropeB{blk_w}")
                    off = q * 4 * blk_w * nblk
                    Q1 = _v(base_t, off, [[blk_w * nblk, 4], [blk_w, nblk], [1, h]])
                    Q2 = _v(base_t, off + h, [[blk_w * nblk, 4], [blk_w, nblk], [1, h]])
                    Av = _v(A, 0, [[nblk * h, 4], [h, nblk], [1, h]])
                    Bvv = _v(Bv, 0, [[nblk * h, 4], [h, nblk], [1, h]])
                    fo = q * 4 * ROT
                    C1 = _v(cos_t, fo, [[ROT, 4], [0, nblk], [1, h]])
                    C2 = _v(cos_t, fo + h, [[ROT, 4], [0, nblk], [1, h]])
                    S1 = _v(sin_t, fo, [[ROT, 4], [0, nblk], [1, h]])
                    S2 = _v(sin_t, fo + h, [[ROT, 4], [0, nblk], [1, h]])
                    eng.tensor_tensor(Av, Q2, S1, OP.mult)
                    eng.tensor_tensor(Bvv, Q1, S2, OP.mult)
                    eng.tensor_tensor(Q1, Q1, C1, OP.mult)
                    eng.tensor_tensor(Q1, Q1, Av, OP.subtract)
                    eng.tensor_tensor(Q2, Q2, C2, OP.mult)
                    eng.tensor_tensor(Q2, Q2, Bvv, OP.add)

                for q in range(4):
                    if q == 0:
                        # first quarter in two half-waves so chunk 0 starts
                        # after ~0.75MB instead of 1.5MB; fr/wo loads deferred
                        # out of the critical initial burst
                        for w in range(2):
                            wsl = slice(w * 256, (w + 1) * 256)
                            for j in range(ECH):
                                nc.sync.dma_start(
                                    out=xt[:, j, wsl],
                                    in_=xT_d[j * 128:(j + 1) * 128, wsl].bitcast(DT))
                            if w == 0:
                                nc.sync.dma_start(out=fr_t, in_=fr_d)
                                # cos/sin tables (emitted after the fr load:
                                # Tile deps follow emission order). fr in
                                # [0,1) so fr + pi/2 stays in [-pi, pi]
                                nc.scalar.activation(cos_t, fr_t, AF.Sin,
                                                     bias=halfpi[:, :], scale=1.0)
                                nc.scalar.activation(sin_t, fr_t, AF.Sin,
                                                     scale=1.0)
                    else:
                        tsl = slice(q * 512, (q + 1) * 512)
                        for j in range(ECH):
                            nc.sync.dma_start(
                                out=xt[:, j, tsl],
                                in_=xT_d[j * 128:(j + 1) * 128, tsl].bitcast(DT))
                    if q == 1:
                        nc.sync.dma_start(out=wo_a, in_=wo_d[0:128, :].bitcast(DT))
                        nc.sync.dma_start(out=wo_b, in_=wo_d[128:192, :].bitcast(DT))
                    for i in range(4 * q, 4 * q + 4):
                        ps_qk = PSC.tile([128, QK], F32, tag="qk")
                        ps_v = PSC.tile([128, 256], F32, tag="v")
                        for j in range(ECH):
                            lhs = xt[:, j, i * 128:(i + 1) * 128]
                            nc.tensor.matmul(ps_qk, lhs, wqk_t[:, j, :],
                                             start=(j == 0), stop=(j == ECH - 1))
                            nc.tensor.matmul(ps_v, lhs, wv_t[:, j, :],
                                             start=(j == 0), stop=(j == ECH - 1))
                        nc.any.tensor_copy(qk_nat[:, i * QK:(i + 1) * QK], ps_qk)
                        # v columns into 65-wide head blocks (ones col untouched)
                        dst = _v(vt, i * VW, [[HD + 1, HPC], [1, HD]])
                        src = _v(ps_v, 0, [[HD, HPC], [1, HD]])
                        nc.any.tensor_copy(dst, src)

                    rope_q(nc.vector, qk_nat, HD, 2 * HPC, q, RS)
                    rope_q(nc.gpsimd, vt, HD + 1, HPC, q, RS)

                    for i in range(4 * q, 4 * q + 4):
                        col = i * QK
                        ts_sl = slice(i * 128, (i + 1) * 128)
                        pt = PSD.tile([128, 512], DT, tag="tr")
                        nc.tensor.transpose(pt[:, 0:128], qk_nat[:, col:col + 128], ident)
                        nc.tensor.transpose(pt[0:64, 128:256], qk_nat[:, col + 128:col + 192], ident)
                        nc.tensor.transpose(pt[:, 256:384], qk_nat[:, col + 192:col + 320], ident)
                        nc.tensor.transpose(pt[0:64, 384:512], qk_nat[:, col + 320:col + 384], ident)
                        nc.any.tensor_copy(qT_a[:, ts_sl], pt[:, 0:128])
                        nc.any.tensor_copy(qT_b[:, ts_sl], pt[0:64, 128:256])
                        nc.any.tensor_copy(kT_a[:, ts_sl], pt[:, 256:384])
                        nc.any.tensor_copy(kT_b[:, ts_sl], pt[0:64, 384:512])

            # ---- phase E: attention per head ----
            with tc.tile_pool(name="psS", bufs=2, space="PSUM") as PSS, \
                 tc.tile_pool(name="psO", bufs=2, space="PSUM") as PSO, \
                 tc.tile_pool(name="epool", bufs=4) as EP, \
                 tc.tile_pool(name="rcp", bufs=4) as RCP:
                for h in range(HPC):
                    if h == 0:
                        qT_h, kT_h = qT_a[0:64, :], kT_a[0:64, :]
                        oT_dst = oT_a[0:64, :]
                    elif h == 1:
                        qT_h, kT_h = qT_a[64:128, :], kT_a[64:128, :]
                        oT_dst = oT_a[64:128, :]
                    else:
                        qT_h, kT_h = qT_b[0:64, :], kT_b[0:64, :]
                        oT_dst = oT_b[0:64, :]

                    for half in range(2):
                        hoff = half * (T // 2)
                        ps_o = PSO.tile([HD + 1, T // 2], F32, tag="po")
                        for m in range(TCH):
                            ps_s = PSS.tile([128, T // 2], F32, tag="ps")
                            for n in range(2):
                                nsl = slice(n * 512, (n + 1) * 512)
                                nc.tensor.matmul(
                                    ps_s[:, nsl],
                                    kT_h[:, m * 128:(m + 1) * 128],
                                    qT_h[:, hoff + n * 512:hoff + (n + 1) * 512],
                                    start=True, stop=True)
                            e_m = EP.tile([128, T // 2], DT, tag="e")
                            nc.scalar.activation(e_m, ps_s, AF.Exp)
                            vh = _v(vt, m * VW + h * (HD + 1), [[1, HD + 1]])
                            for n in range(2):
                                nc.tensor.matmul(
                                    ps_o[:, n * 512:(n + 1) * 512],
                                    vh, e_m[:, n * 512:(n + 1) * 512],
                                    start=(m == 0), stop=(m == TCH - 1))

                        # per-half deferred normalization: copy raw out.T and
                        # reciprocal of the ones-row denominator, freeing ps_o
                        # early; the partition-broadcast goes through a DRAM
                        # bounce buffer.
                        hsl = slice(hoff, hoff + T // 2)
                        recip_h = RCP.tile([1, T // 2], F32, tag="rc")
                        nc.vector.reciprocal(recip_h, ps_o[HD:HD + 1, :])
                        nc.vector.tensor_copy(oT_dst[:, hsl], ps_o[0:HD, :])
                        nc.sync.dma_start(out=dr_bounce[h:h + 1, hsl], in_=recip_h)
                        # broadcast into the partition range matching oT_dst's
                        # base partition (DVE needs equal SB base partitions)
                        pbase = 64 * (h % 2)
                        rb_h = RCP.tile([128, T // 2], F32, tag="rb")
                        rb_src = bass.AP(tensor=dr_bounce.tensor,
                                         offset=dr_bounce.offset + h * T + hoff,
                                         ap=[[0, 64], [1, T // 2]])
                        nc.sync.dma_start(out=rb_h[pbase:pbase + 64, :], in_=rb_src)
                        nc.vector.tensor_tensor(oT_dst[:, hsl], oT_dst[:, hsl],
                                                rb_h[pbase:pbase + 64, :], OP.mult)

            # ---- phase F: output projection ----
            with tc.tile_pool(name="psY", bufs=4, space="PSUM") as PSY, \
                 tc.tile_pool(name="ysb", bufs=4) as YSB:
                for m in range(TCH):
                    msl = slice(m * 128, (m + 1) * 128)
                    ps_y = PSY.tile([128, E], F32, tag="y")
                    for nsl in (slice(0, 512), slice(512, E)):
                        nc.tensor.matmul(ps_y[:, nsl], oT_a[:, msl],
                                         wo_a[:, nsl], start=True, stop=False)
                        nc.tensor.matmul(ps_y[:, nsl], oT_b[:, msl],
                                         wo_b[:, nsl], start=False, stop=True)
                    y_s = YSB.tile([128, E], F32, tag="ys")
                    nc.vector.tensor_copy(y_s[:, 0:384], ps_y[:, 0:384])
                    nc.scalar.copy(y_s[:, 384:E], ps_y[:, 384:E])
                    nc.sync.dma_start(out=y_d[msl, :], in_=y_s)

    nc.compile()
    return nc


_NC_CACHE = None


def _get_nc():
    global _NC_CACHE
    if _NC_CACHE is None:
        _NC_CACHE = build_nc()
    return _NC_CACHE


def make_in_maps(hidden_states, rotary_pos_emb, Wq, Wk, Wv, Wo):
    fr = np.ascontiguousarray(
        rotary_pos_emb.reshape(TCH, 128, ROT).transpose(1, 0, 2).reshape(128, TCH * ROT),
        dtype=np.float32)
    in_maps = []
    for c in range(NC_CORES):
        b = c // (NC_CORES // B)
        g = c % (NC_CORES // B)
        rows = slice(HPC * HD * g, HPC * HD * (g + 1))
        xT = np.ascontiguousarray(hidden_states[b].T, dtype=np.float32)
        wqk = np.concatenate([Wq[rows].T * SCALE, Wk[rows].T], axis=1)
        wv = np.zeros((E, 256), dtype=np.float32)
        wv[:, :HPC * HD] = Wv[rows].T
        wo = np.ascontiguousarray(Wo[:, rows].T, dtype=np.float32)
        in_maps.append({
            "xT": xT,
            "wqk": np.ascontiguousarray(wqk, dtype=np.float32),
            "wv": wv,
            "wo": wo,
            "fr": fr,
        })
    return in_maps


def kernel(hidden_states, rotary_pos_emb, Wq, Wk, Wv, Wo, bo):
    hidden_states = np.asarray(hidden_states, dtype=np.float32)
    rotary_pos_emb = np.asarray(rotary_pos_emb, dtype=np.float32)
    Wq = np.asarray(Wq, dtype=np.float32)
    Wk = np.asarray(Wk, dtype=np.float32)
    Wv = np.asarray(Wv, dtype=np.float32)
    Wo = np.asarray(Wo, dtype=np.float32)
    bo = np.asarray(bo, dtype=np.float32)

    from concourse import bass_utils as _bu
    nc = _get_nc()
    in_maps = make_in_maps(hidden_states, rotary_pos_emb.reshape(T, ROT),
                           Wq, Wk, Wv, Wo)
    res = _bu.run_bass_kernel_spmd(
        nc, in_maps, core_ids=list(range(NC_CORES)), trace=False)

    out = np.zeros((B, T, E), dtype=np.float32)
    for c in range(NC_CORES):
        out[c // (NC_CORES // B)] += res.results[c]["y"]
    out += bo
    return out
